# revision 10
# baseline (speedup 1.0000x reference)
"""Trainium2 Bass kernel for nn_D_loss_67551245631962.

Computes: 0.8 * sum(WMA5(target_angle - pred_angle)^2) + 0.2 * sum((target_class - pred_class)^2)
where WMA5 is a 5-tap [0.05, 0.1, 0.7, 0.1, 0.05] correlation with 2-zero padding per side.

Strategy (pure data parallelism over batch dim B=2048 across 8 cores, 256 rows/core):
  - Both angle inputs stream in via SWDGE cast DMAs (fp32 DRAM -> fp16 SBUF).
    Mixed tile widths per 128-row group: [2048,2048,2048,1024,512,512] - wide
    tiles give 8KB DMA descriptors (best per-byte queue efficiency), narrow
    tiles at the end keep the post-last-load serial tail short.
  - DVE (3 passes/tile, fp16 2x): dbf = ta - pa; u = d1 + d3; v = d0 + d4.
  - PE (otherwise idle) assembles the conv via scaled-identity matmuls
    accumulated in PSUM per 512-col bank chunk: psum = 14I@d2 + 2I@u + I@v,
    i.e. s4 = 14*d2 + 2*(d1+d3) + (d0+d4) = wma/0.05.
  - ACT squares PSUM directly with accum_out into per-tile partial columns.
  - Host sums 8 cores' [128, NACC] partials in float64, scales 0.8*0.05^2 / 0.2.
  Engine budget/core: DMA ~41us queue work (16.8 MB HBM at ~420 GB/s agg),
  DVE ~31us, PE ~32us, ACT ~21us -> memory-bound.
"""

import os
import sys

# v2 ASAP tile scheduler beats the legacy CoreSim-schedule flow by ~11% here;
# must be set before concourse.env caches the value.
os.environ.setdefault("TILE_SCHEDULER", "asap")

for _p in ("/opt/trn_rl_repo",):
    if os.path.isdir(_p) and _p not in sys.path:
        sys.path.insert(0, _p)

from contextlib import ExitStack

import numpy as np

import concourse.bass as bass
import concourse.tile as tile
from concourse import bacc, mybir
from concourse.bass_utils import run_bass_kernel_spmd

N_CORES = 8
B, T = 2048, 8192
RPC = B // N_CORES  # rows per core = 256
G = RPC // 128      # 128-partition row groups per core = 2
WIDTHS = [2048, 2048, 2048, 1024, 512, 512]  # per-group column tile widths
assert sum(WIDTHS) == T
NT = len(WIDTHS)
NTILES = G * NT     # angle tiles per core = 12
NACC = NTILES + G   # accumulator columns: angle tiles + class groups
FMAX = max(WIDTHS)
CH = 512            # PSUM bank chunk (fp32 cols per bank)

W = (0.05, 0.1, 0.7, 0.1, 0.05)

DT16 = mybir.dt.float16  # conv compute storage dtype (2-byte => DVE 2x mode)


def build_nc():
    nc = bacc.Bacc("TRN2")
    dt = mybir.dt
    ta = nc.dram_tensor("target_angle", [RPC, T], dt.float32, kind="ExternalInput")
    pa = nc.dram_tensor("pred_angle", [RPC, T], dt.float32, kind="ExternalInput")
    tcl = nc.dram_tensor("target_class", [RPC, 3], dt.float32, kind="ExternalInput")
    pcl = nc.dram_tensor("pred_class", [RPC, 3], dt.float32, kind="ExternalInput")
    out = nc.dram_tensor("out", [128, NACC], dt.float32, kind="ExternalOutput")

    AF = mybir.ActivationFunctionType
    OP = mybir.AluOpType

    starts = [sum(WIDTHS[:t]) for t in range(NT)]

    # tile i covers rows [g*128,(g+1)*128), diff cols [c0-2, c0+Fi+2) w/ halo
    def geom(i):
        g, t = divmod(i, NT)
        c0, Fi = starts[t], WIDTHS[t]
        lo, hi = c0 - 2, c0 + Fi + 2
        dst_lo, dst_hi = 0, Fi + 4
        if lo < 0:
            dst_lo, lo = 2, 0
        if hi > T:
            dst_hi, hi = Fi + 2, T
        return g * 128, (g + 1) * 128, lo, hi, dst_lo, dst_hi, Fi

    with tile.TileContext(nc) as tc, ExitStack() as ctx:
        tpool = ctx.enter_context(tc.tile_pool(name="dta", bufs=NTILES))
        qpool = ctx.enter_context(tc.tile_pool(name="dpa", bufs=NTILES))
        wkpool = ctx.enter_context(tc.tile_pool(name="wk", bufs=12))
        ppool = ctx.enter_context(tc.tile_pool(name="ps", bufs=2, space="PSUM"))
        apool = ctx.enter_context(tc.tile_pool(name="acc", bufs=1))
        cpool = ctx.enter_context(tc.tile_pool(name="cls", bufs=8))
        wpool = ctx.enter_context(tc.tile_pool(name="wid", bufs=1))

        accums = apool.tile([128, NACC], dt.float32)

        # scaled-identity stationaries for the PE conv assembly
        def make_diag(scale, name):
            m = wpool.tile([128, 128], DT16, name=f"m_{name}")
            nc.gpsimd.memset(m[:], scale)
            s = wpool.tile([128, 128], DT16, name=f"id_{name}")
            nc.gpsimd.affine_select(
                s[:], m[:], [[1, 128]], OP.is_equal, 0.0,
                base=0, channel_multiplier=-1,
            )
            return s

        dtas = [
            tpool.tile([128, geom(i)[6] + 4], DT16, tag="dta", name=f"dta{i}")
            for i in range(NTILES)
        ]
        dpas = [
            qpool.tile([128, geom(i)[6] + 4], DT16, tag="dpa", name=f"dpa{i}")
            for i in range(NTILES)
        ]
        for i in range(NTILES):
            _, _, _, _, dst_lo, dst_hi, Fi = geom(i)
            if dst_lo:
                nc.vector.memset(dtas[i][:, 0:dst_lo], 0.0)
                nc.vector.memset(dpas[i][:, 0:dst_lo], 0.0)
            if dst_hi < Fi + 4:
                nc.vector.memset(dtas[i][:, dst_hi : Fi + 4], 0.0)
                nc.vector.memset(dpas[i][:, dst_hi : Fi + 4], 0.0)

        # class loads early on HWDGE (sync) - free, doesn't touch gpsimd stream
        ctls, cpls = [], []
        for g in range(G):
            r0, r1_ = g * 128, (g + 1) * 128
            ct = cpool.tile([128, 3], dt.float32, tag="cls")
            cp = cpool.tile([128, 3], dt.float32, tag="clsp")
            nc.sync.dma_start(ct[:], tcl[r0:r1_, :])
            nc.sync.dma_start(cp[:], pcl[r0:r1_, :])
            ctls.append(ct)
            cpls.append(cp)

        id14 = id2 = id1 = None
        for i in range(NTILES):
            r0, r1_, lo, hi, dst_lo, dst_hi, Fi = geom(i)
            dta, dpa = dtas[i], dpas[i]
            nc.gpsimd.dma_start(dta[:, dst_lo:dst_hi], ta[r0:r1_, lo:hi])
            nc.gpsimd.dma_start(dpa[:, dst_lo:dst_hi], pa[r0:r1_, lo:hi])
            if i == 0:
                # diags sit in the gpsimd stream right after tile 0's loads:
                # ready (~3us) well before PE's first matmul needs them.
                id14 = make_diag(14.0, "w14")
                id2 = make_diag(2.0, "w2")
                id1 = make_diag(1.0, "w1")

            dbf = wkpool.tile([128, FMAX + 4], DT16, tag="wk", name=f"dbf{i}")
            nc.vector.tensor_sub(dbf[:, 0 : Fi + 4], dta[:], dpa[:])
            u = wkpool.tile([128, FMAX + 4], DT16, tag="wk", name=f"u{i}")
            nc.vector.tensor_add(u[:, 0:Fi], dbf[:, 1 : Fi + 1], dbf[:, 3 : Fi + 3])
            v = wkpool.tile([128, FMAX + 4], DT16, tag="wk", name=f"v{i}")
            nc.vector.tensor_add(v[:, 0:Fi], dbf[:, 0:Fi], dbf[:, 4 : Fi + 4])

            psum = ppool.tile([128, FMAX], dt.float32, tag="ps")
            for c in range(Fi // CH):
                sl = slice(c * CH, (c + 1) * CH)
                nc.tensor.matmul(
                    psum[:, sl], id14, dbf[:, 2 + c * CH : 2 + (c + 1) * CH],
                    start=True, stop=False,
                )
                nc.tensor.matmul(
                    psum[:, sl], id2, u[:, sl], start=False, stop=False
                )
                nc.tensor.matmul(
                    psum[:, sl], id1, v[:, sl], start=False, stop=True
                )

            nc.scalar.activation(
                psum[:, 0:Fi], psum[:, 0:Fi], AF.Square,
                accum_out=accums[:, i : i + 1],
            )

            if i == 2:
                # class SSE per row group (tiny); emitted mid-stream so it
                # never sits on the kernel's tail
                for g in range(G):
                    cd = cpool.tile([128, 3], dt.float32, name=f"cd{g}")
                    nc.vector.tensor_sub(cd[:], ctls[g][:], cpls[g][:])
                    cj = cpool.tile([128, 3], dt.float32, name=f"cj{g}")
                    col = NTILES + g
                    nc.scalar.activation(
                        cj[:], cd[:], AF.Square, accum_out=accums[:, col : col + 1]
                    )

        nc.sync.dma_start(out[:], accums[:])

    nc.finalize()
    return nc


_NC = None
last_result = None  # BassKernelResults of the most recent run (for test harness)


def kernel(target_angle, pred_angle, target_class, pred_class):
    global _NC, last_result
    if _NC is None:
        _NC = build_nc()

    in_maps = []
    for c in range(N_CORES):
        r = slice(c * RPC, (c + 1) * RPC)
        in_maps.append(
            {
                "target_angle": np.ascontiguousarray(target_angle[r], dtype=np.float32),
                "pred_angle": np.ascontiguousarray(pred_angle[r], dtype=np.float32),
                "target_class": np.ascontiguousarray(target_class[r], dtype=np.float32),
                "pred_class": np.ascontiguousarray(pred_class[r], dtype=np.float32),
            }
        )

    last_result = run_bass_kernel_spmd(
        _NC,
        in_maps,
        core_ids=list(range(N_CORES)),
        trace=bool(os.environ.get("BASS_TRACE")),
    )

    angle = 0.0
    cls = 0.0
    for r in last_result.results:
        o = np.asarray(r["out"], dtype=np.float64)
        angle += o[:, 0:NTILES].sum()
        cls += o[:, NTILES:NACC].sum()

    val = 0.8 * (W[4] * W[4]) * angle + 0.2 * cls
    return np.array(val, dtype=np.float32)
